# revision 2
# baseline (speedup 1.0000x reference)
"""Masked multi-head attention on 8 TRN2 NeuronCores.

Sharding: 8 cores = 2 batches x 4 head-groups (4 heads of 64 dims each).
Each core computes full causal attention for its (batch, 4-head) slice:
  Q^T/K^T projections (dh on partitions), V in k-major layout augmented with a
  ones column (PV matmul then yields numerator rows 0-63 and the softmax
  denominator in row 64 of one PSUM accumulation), scores S^T = K^T-tile.T @ Q^T
  per 128-key tile with causal tiles skipped and diagonal tiles
  column-restricted + additively masked, exp on ScalarE (no max subtraction:
  scores are ~N(0,1) so exp never overflows), denominator broadcast via a K=1
  matmul, fast-reciprocal and multiply. All matmuls run in float32r (full-rate
  fp32 mode, ~1e-4 rounding). Output is attn^T per core; host transposes and
  concatenates.
"""
import threading
from contextlib import ExitStack

import numpy as np

import concourse.bass as bass
import concourse.tile as tile
from concourse import bacc, mybir
from concourse.bass_utils import run_bass_kernel_spmd

F32 = mybir.dt.float32
F32R = mybir.dt.float32r

B, T, C = 2, 2048, 1024
H, DH = 16, 64
HPC = 4            # heads per core
RPC = HPC * DH     # 256 output channels per core
NCT = C // 128     # 8 contraction tiles
NQC = T // 512     # 4 query chunks
NKT = T // 128     # 16 key tiles
NEG = -1.0e30


def _build(n_iter: int = 1):
    nc = bacc.Bacc("TRN2", target_bir_lowering=False, debug=False)
    xt = nc.dram_tensor("xt", [C, T], F32R, kind="ExternalInput").ap()
    wq = nc.dram_tensor("wq", [C, RPC], F32R, kind="ExternalInput").ap()
    wk = nc.dram_tensor("wk", [C, RPC], F32R, kind="ExternalInput").ap()
    wv = nc.dram_tensor("wv", [C, RPC], F32R, kind="ExternalInput").ap()
    bq = nc.dram_tensor("bq", [1, RPC], F32R, kind="ExternalInput").ap()
    bk = nc.dram_tensor("bk", [1, RPC], F32R, kind="ExternalInput").ap()
    bv = nc.dram_tensor("bv", [1, RPC], F32R, kind="ExternalInput").ap()
    mask = nc.dram_tensor("mask", [128, 128], F32, kind="ExternalInput").ap()
    ones = nc.dram_tensor("ones", [128, 512], F32R, kind="ExternalInput").ap()
    ot = nc.dram_tensor("ot", [RPC, T], F32, kind="ExternalOutput").ap()

    with tile.TileContext(nc) as tc, ExitStack() as ctx:
        if n_iter > 1:
            ctx.enter_context(tc.For_i(0, n_iter))
        per = ctx.enter_context(tc.tile_pool(name="per", bufs=1))
        wrk = ctx.enter_context(tc.tile_pool(name="wrk", bufs=3))
        tl = ctx.enter_context(tc.tile_pool(name="tl", bufs=2))
        ps = ctx.enter_context(tc.tile_pool(name="ps", bufs=1, space="PSUM"))

        # ---- load phase ----
        xt_s = per.tile([128, NCT, T], F32R, tag="xt")
        for ct in range(NCT):
            nc.sync.dma_start(xt_s[:, ct, :], xt[128 * ct:128 * (ct + 1), :])
        wq_s = per.tile([128, NCT, RPC], F32R, tag="wq")
        wk_s = per.tile([128, NCT, RPC], F32R, tag="wk")
        wv_s = per.tile([128, NCT, RPC], F32R, tag="wv")
        nc.sync.dma_start(wq_s[:], wq.rearrange("(c p) m -> p c m", p=128))
        nc.sync.dma_start(wk_s[:], wk.rearrange("(c p) m -> p c m", p=128))
        nc.sync.dma_start(wv_s[:], wv.rearrange("(c p) m -> p c m", p=128))
        bq_s = per.tile([1, RPC], F32R, tag="bq")
        bk_s = per.tile([1, RPC], F32R, tag="bk")
        bv_s = per.tile([1, RPC], F32R, tag="bv")
        nc.sync.dma_start(bq_s[:], bq[:])
        nc.sync.dma_start(bk_s[:], bk[:])
        nc.sync.dma_start(bv_s[:], bv[:])
        mask_s = per.tile([128, 128], F32, tag="mask")
        nc.sync.dma_start(mask_s[:], mask[:])
        ones_s = per.tile([128, 512], F32R, tag="ones")
        nc.sync.dma_start(ones_s[:], ones[:])

        # V augmented with a ones column: [k-part, ktile, head, 65]
        v_aug = per.tile([128, NKT, HPC, DH + 1], F32R, tag="vaug")
        nc.sync.dma_start(
            v_aug[:, :, :, DH],
            ones_s[:, 0:NKT * HPC].rearrange("p (a b) -> p a b", a=NKT),
        )

        # ---- projections ----
        qt_s = per.tile([128, 2, T], F32R, tag="qt")
        kt_s = per.tile([128, 2, T], F32R, tag="kt")
        for w_s, b_s, o_s in ((wq_s, bq_s, qt_s), (wk_s, bk_s, kt_s)):
            for gr in range(2):
                for chk in range(NQC):
                    pq = ps.tile([128, 512], F32, tag="pp")
                    for ct in range(NCT):
                        nc.tensor.matmul(
                            pq[:],
                            w_s[:, ct, 128 * gr:128 * (gr + 1)],
                            xt_s[:, ct, 512 * chk:512 * (chk + 1)],
                            start=(ct == 0), stop=False,
                        )
                    nc.tensor.matmul(
                        pq[:],
                        b_s[0:1, 128 * gr:128 * (gr + 1)],
                        ones_s[0:1, :],
                        start=False, stop=True,
                    )
                    nc.vector.tensor_copy(
                        o_s[:, gr, 512 * chk:512 * (chk + 1)], pq[:])
        for tt in range(NKT):
            pv = ps.tile([128, RPC], F32, tag="pp")
            for ct in range(NCT):
                nc.tensor.matmul(
                    pv[:],
                    xt_s[:, ct, 128 * tt:128 * (tt + 1)],
                    wv_s[:, ct, :],
                    start=(ct == 0), stop=False,
                )
            nc.tensor.matmul(
                pv[:], ones_s[0:1, 0:128], bv_s[:], start=False, stop=True)
            nc.vector.tensor_copy(
                v_aug[:, tt, :, 0:DH],
                pv[:].rearrange("p (h d) -> p h d", h=HPC),
            )

        # ---- attention ----
        for h in range(HPC):
            gr, p0 = h // 2, 64 * (h % 2)
            ot_h = tl.tile([64, T], F32, tag="ot")
            for chk in range(NQC):
                q0 = 512 * chk
                nt = ps.tile([DH + 1, 512], F32, tag="nt")
                # full (strictly below-diagonal) key tiles, in pairs
                for pr in range(2 * chk):
                    s2 = ps.tile([128, 1024], F32, tag="s2")
                    e2 = wrk.tile([128, 1024], F32R, tag="e2")
                    for j in range(2):
                        kt = 2 * pr + j
                        nc.tensor.matmul(
                            s2[:, 512 * j:512 * (j + 1)],
                            kt_s[p0:p0 + 64, gr, 128 * kt:128 * (kt + 1)],
                            qt_s[p0:p0 + 64, gr, q0:q0 + 512],
                            start=True, stop=True,
                        )
                    nc.scalar.activation(
                        e2[:], s2[:], mybir.ActivationFunctionType.Exp)
                    for j in range(2):
                        kt = 2 * pr + j
                        nc.tensor.matmul(
                            nt[:],
                            v_aug[:, kt, h, :],
                            e2[:, 512 * j:512 * (j + 1)],
                            start=(kt == 0), stop=False,
                            skip_group_check=True,
                        )
                # diagonal key tiles (column-restricted, causally masked)
                for m in range(4):
                    kt = 4 * chk + m
                    w0 = 128 * m
                    sd = ps.tile([128, 512], F32, tag="pp")
                    ed = wrk.tile([128, 512], F32R, tag="ed")
                    nc.tensor.matmul(
                        sd[:, w0:512],
                        kt_s[p0:p0 + 64, gr, 128 * kt:128 * (kt + 1)],
                        qt_s[p0:p0 + 64, gr, q0 + w0:q0 + 512],
                        start=True, stop=True,
                    )
                    nc.vector.tensor_add(
                        sd[:, w0:w0 + 128], sd[:, w0:w0 + 128], mask_s[:])
                    nc.scalar.activation(
                        ed[:, w0:512], sd[:, w0:512],
                        mybir.ActivationFunctionType.Exp)
                    nc.tensor.matmul(
                        nt[:, w0:512],
                        v_aug[:, kt, h, :],
                        ed[:, w0:512],
                        start=(chk == 0 and m == 0), stop=(m == 3),
                        skip_group_check=True,
                    )
                # normalize: row 64 of nt holds the softmax denominator
                dr = tl.tile([1, 512], F32R, tag="dr")
                nc.vector.tensor_copy(dr[:], nt[DH:DH + 1, :])
                rbp = ps.tile([64, 512], F32, tag="rb")
                nc.tensor.matmul(
                    rbp[:], ones_s[0:1, 0:64], dr[:], start=True, stop=True)
                rb = tl.tile([64, 512], F32, tag="rbs")
                nc.vector.reciprocal_approx_fast(out=rb[:], in_=rbp[:])
                nc.vector.tensor_mul(ot_h[:, q0:q0 + 512], nt[0:DH, :], rb[:])
            nc.sync.dma_start(ot[64 * h:64 * (h + 1), :], ot_h[:])

    nc.compile()
    return nc


_LOCK = threading.Lock()
_NC = None


def _get_nc():
    global _NC
    with _LOCK:
        if _NC is None:
            _NC = _build()
    return _NC


def _causal_mask_tile():
    kp = np.arange(128)[:, None]
    j = np.arange(128)[None, :]
    return np.where(j >= kp, 0.0, NEG).astype(np.float32)


def kernel(X, Wq, bq, Wk, bk, Wv, bv):
    X = np.asarray(X, dtype=np.float32)
    Wq = np.asarray(Wq, dtype=np.float32)
    Wk = np.asarray(Wk, dtype=np.float32)
    Wv = np.asarray(Wv, dtype=np.float32)
    bq = np.asarray(bq, dtype=np.float32)
    bk = np.asarray(bk, dtype=np.float32)
    bv = np.asarray(bv, dtype=np.float32)

    nc = _get_nc()
    s = 1.0 / np.sqrt(DH).astype(np.float32)
    mask = _causal_mask_tile()
    ones = np.ones((128, 512), dtype=np.float32)
    in_maps = []
    for core in range(8):
        b, g = divmod(core, 4)
        r0 = RPC * g
        sl = slice(r0, r0 + RPC)
        in_maps.append({
            "xt": np.ascontiguousarray(X[b].T),
            "wq": np.ascontiguousarray((Wq[sl] * s).T),
            "wk": np.ascontiguousarray(Wk[sl].T),
            "wv": np.ascontiguousarray(Wv[sl].T),
            "bq": (bq[sl] * s).reshape(1, RPC),
            "bk": bk[sl].reshape(1, RPC),
            "bv": bv[sl].reshape(1, RPC),
            "mask": mask,
            "ones": ones,
        })
    res = run_bass_kernel_spmd(nc, in_maps, core_ids=list(range(8)))
    out = np.empty((B, T, C), dtype=np.float32)
    for core in range(8):
        b, g = divmod(core, 4)
        out[b, :, RPC * g:RPC * (g + 1)] = res.results[core]["ot"].T
    return out
